# revision 1
# baseline (speedup 1.0000x reference)
# DenseGATConv on 8 Trainium2 NeuronCores (Bass/Tile, SPMD over destination rows).
#
# Math: h = x@W ; el/er = head-wise <h, att> ; e_ij = leaky(el_i + er_j) ;
#       alpha = softmax_j(mask(e)) ; out_i = sum_j alpha_ij h_j + bias.
# Key identity: exp(leaky(s)) = max(exp(s), exp(0.2 s)) since exp is monotone
# and leaky(s) = max(s, 0.2 s).  With s_ij = el_i + er_j both branches are
# rank-1 outer products: exp(s) = exp(el_i) exp(er_j).  The masked unnormalized
# attention is  pm[j,i] = adj[i,j] * max(al_i*ar_j, bl_i*br_j)  which needs no
# transcendentals on the [N,N,H] tensor — just two fused DVE ops + a max.
# The denominator rides along as a ones-column in the aggregation matmul.
#
# Sharding: destination rows i split across 8 cores (512 rows each); every core
# computes the full h (it needs all source nodes j anyway); params replicated.
import numpy as np

N, IN_C, HEADS, OUT_C = 4096, 256, 4, 64
HC = HEADS * OUT_C          # 256
NCORES = 8
NB = N // NCORES            # 512 destination rows per core
JT = N // 128               # 32 source-node tiles
IT = NB // 128              # 4 row subtiles per core
C65 = OUT_C + 1             # head slice + ones column

TRACE = False               # test.py flips this to collect HW exec time
LAST_RESULTS = {}           # exec_time_ns etc. stashed here when TRACE

_compiled = {}


def _emit(ctx, tc, nc, io):
    import concourse.bass as bass
    import concourse.masks as masks
    from concourse import mybir

    dt = mybir.dt
    Alu = mybir.AluOpType
    Act = mybir.ActivationFunctionType

    xT, xoT, adjbT, Waug, Wal, bias, out = (
        io["xT"], io["xoT"], io["adjbT"], io["Waug"], io["Wal"],
        io["bias"], io["out"],
    )

    big = ctx.enter_context(tc.tile_pool(name="big", bufs=1))
    tr = ctx.enter_context(tc.tile_pool(name="tr", bufs=3))
    adjpool = ctx.enter_context(tc.tile_pool(name="adjpool", bufs=2))
    ps = ctx.enter_context(tc.tile_pool(name="ps", bufs=2, space="PSUM"))
    pf = ctx.enter_context(tc.tile_pool(name="pf", bufs=1, space="PSUM"))
    pacc = ctx.enter_context(tc.tile_pool(name="pacc", bufs=1, space="PSUM"))

    # ---- constants / params -------------------------------------------------
    idf = big.tile([128, 128], dt.float32, tag="idf")
    masks.make_identity(nc, idf[:])
    idb = big.tile([128, 128], dt.bfloat16, tag="idb")
    masks.make_identity(nc, idb[:])
    bias_b = big.tile([128, HC], dt.float32, tag="bias_b")
    bias_bcast_ap = bass.AP(
        tensor=bias.tensor, offset=bias.offset, ap=[[0, 128]] + list(bias.ap)
    )
    nc.gpsimd.dma_start(out=bias_b[:], in_=bias_bcast_ap)

    waug = []
    wal = []
    for ct in range(2):
        wg = big.tile([128, HC + HEADS], dt.float32r, tag=f"waug{ct}")
        nc.sync.dma_start(out=wg[:], in_=Waug[ct * 128:(ct + 1) * 128, :])
        waug.append(wg)
        wl = big.tile([128, HEADS], dt.float32, tag=f"wal{ct}")
        nc.sync.dma_start(out=wl[:], in_=Wal[ct * 128:(ct + 1) * 128, :])
        wal.append(wl)

    xTr = []
    for ct in range(2):
        xf = big.tile([128, N], dt.float32r, tag=f"xTr{ct}")
        nc.sync.dma_start(out=xf[:], in_=xT[ct * 128:(ct + 1) * 128, :])
        xTr.append(xf)
    xo = []
    for ct in range(2):
        t = big.tile([128, NB], dt.float32, tag=f"xoT{ct}")
        nc.sync.dma_start(out=t[:], in_=xoT[ct * 128:(ct + 1) * 128, :])
        xo.append(t)

    # ---- h65 (bf16 h + ones col) and er via one augmented matmul ------------
    # er_pack laid out h-major (col = h*32 + nt) so a PE transpose yields each
    # head's exp(er) as a 32-aligned partition block.
    h65 = []
    arh65 = []
    er_pack = big.tile([128, JT * HEADS], dt.float32, tag="er_pack")
    ar_pack = big.tile([128, JT * HEADS], dt.float32, tag="ar_pack")
    br_pack = big.tile([128, JT * HEADS], dt.float32, tag="br_pack")
    erp = er_pack[:].rearrange("p (h j) -> p h j", h=HEADS)
    for nt in range(JT):
        hps = ps.tile([128, HC + HEADS], dt.float32, tag="scr")
        for ct in range(2):
            nc.tensor.matmul(
                hps[:], lhsT=xTr[ct][:, nt * 128:(nt + 1) * 128], rhs=waug[ct][:],
                start=(ct == 0), stop=(ct == 1),
            )
        ht = big.tile([128, HEADS * C65], dt.bfloat16, tag=f"h65_{nt}")
        hr = ht[:].rearrange("p (h c) -> p h c", c=C65)
        hpr = hps[:, 0:HC].rearrange("p (h c) -> p h c", c=OUT_C)
        if nt % 2 == 0:
            nc.scalar.copy(hr[:, :, 0:OUT_C], hpr[:, :, :])
        else:
            nc.vector.tensor_copy(hr[:, :, 0:OUT_C], hpr[:, :, :])
        nc.vector.memset(hr[:, :, OUT_C], 1.0)
        h65.append(ht)
        nc.any.tensor_copy(erp[:, :, nt], hps[:, HC:HC + HEADS])
        if nt % 8 == 7:
            # exp the finished chunk: cols h*32+nt for nt in chunk, all h
            for h in range(HEADS):
                c0, c1 = h * JT + nt - 7, h * JT + nt + 1
                nc.scalar.activation(ar_pack[:, c0:c1], er_pack[:, c0:c1], Act.Exp)
                nc.scalar.activation(br_pack[:, c0:c1], er_pack[:, c0:c1],
                                     Act.Exp, scale=0.2)
            # ar-scaled copies of h65 (ar in the ones column -> denominator)
            for nt2 in range(nt - 7, nt + 1):
                at = big.tile([128, HEADS * C65], dt.bfloat16, tag=f"arh65_{nt2}")
                for h in range(HEADS):
                    sc = ar_pack[:, h * JT + nt2:h * JT + nt2 + 1]
                    if h < 2:
                        nc.scalar.activation(
                            at[:, h * C65:(h + 1) * C65],
                            h65[nt2][:, h * C65:(h + 1) * C65], Act.Copy, scale=sc,
                        )
                    else:
                        nc.vector.tensor_scalar_mul(
                            at[:, h * C65:(h + 1) * C65],
                            h65[nt2][:, h * C65:(h + 1) * C65], sc,
                        )
                arh65.append(at)

    # transposed exp(er) rows per head: [2, N] bf16 (row0=br, row1=ar)
    arb16 = big.tile([128, JT * HEADS], dt.bfloat16, tag="arb16")
    brb16 = big.tile([128, JT * HEADS], dt.bfloat16, tag="brb16")
    nc.vector.tensor_copy(arb16[:], ar_pack[:])
    nc.vector.tensor_copy(brb16[:], br_pack[:])
    arT_ps = ps.tile([128, 128], dt.bfloat16, tag="scr")
    brT_ps = ps.tile([128, 128], dt.bfloat16, tag="scr")
    nc.tensor.transpose(arT_ps[:], arb16[:], idb[:])
    nc.tensor.transpose(brT_ps[:], brb16[:], idb[:])
    arT_sb = big.tile([128, 128], dt.bfloat16, tag="arT_sb")
    brT_sb = big.tile([128, 128], dt.bfloat16, tag="brT_sb")
    nc.vector.tensor_copy(arT_sb[:], arT_ps[:])
    nc.vector.tensor_copy(brT_sb[:], brT_ps[:])
    arbr = []
    for h in range(HEADS):
        t = big.tile([2, N], dt.bfloat16, tag=f"arbr_{h}", name=f"arbr_{h}")
        nc.sync.dma_start(out=t[0:1, :], in_=brT_sb[h * JT:(h + 1) * JT, :])
        nc.sync.dma_start(out=t[1:2, :], in_=arT_sb[h * JT:(h + 1) * JT, :])
        arbr.append(t)

    # ---- el side: exp rows + d-matmul rhs + al broadcast --------------------
    d_rhs = []
    al_rows = []
    for h in range(HEADS):
        elp = ps.tile([1, NB], dt.float32, tag="scr")
        for ct in range(2):
            nc.tensor.matmul(
                elp[:], lhsT=wal[ct][:, h:h + 1], rhs=xo[ct][:],
                start=(ct == 0), stop=(ct == 1),
            )
        dr = big.tile([2, NB], dt.bfloat16, tag=f"d_rhs_{h}", name=f"d_rhs_{h}")
        # row0 = bl = exp(0.2 el) directly from ACT (partition 0 ok)
        nc.scalar.activation(dr[0:1, :], elp[:], Act.Exp, scale=0.2)
        al_row = big.tile([1, NB], dt.float32, tag=f"al_row_{h}")
        nc.scalar.activation(al_row[:], elp[:], Act.Exp)
        al_rows.append(al_row)
        nal = big.tile([1, NB], dt.bfloat16, tag=f"nal_{h}")
        nc.vector.tensor_scalar_mul(nal[:], al_row[:], -1.0)
        nc.sync.dma_start(out=dr[1:2, :], in_=nal[:])
        d_rhs.append(dr)

    al_cols = []
    for it in range(IT):
        t = big.tile([128, HEADS], dt.float32, tag=f"al_cols_{it}")
        for h in range(HEADS):
            nc.sync.dma_start(
                out=t[:, h:h + 1],
                in_=al_rows[h][0:1,
                    it * 128:(it + 1) * 128],
            )
        al_cols.append(t)

    # ---- adjacency: host-pretransposed [N, NB]; load + cast bf16 (0/1) ------
    adjT = []
    for jt in range(JT):
        ai = adjpool.tile([128, NB], dt.int32, tag="adjint")
        nc.sync.dma_start(out=ai[:], in_=adjbT[jt * 128:(jt + 1) * 128, :])
        ab = big.tile([128, NB], dt.bfloat16, tag=f"adjT{jt}", name=f"adjT{jt}")
        nc.vector.tensor_copy(ab[:], ai[:])
        adjT.append(ab)

    # ---- main loops: two head-passes (PSUM budget), mask rides the matmuls --
    # pm = m*A + m*relu(B-A):  the m*A term is a pure matmul (lhsT = ar-scaled
    # h65, rhs = adjacency); d = B-A comes from a K=2 rank-2 matmul; the only
    # per-element vector op is r = relu(d)*m (fused scalar_tensor_tensor).
    # flipped agg1: out[i, (h,c)] accumulators, one wide matmul per (jt, it);
    # lhsT = adjacency tile (i-slice), rhs = ar-scaled h65 for all heads.
    po1f = [pf.tile([128, HEADS * C65], dt.float32, name=f"po1f_{it}",
                    tag=f"po1f_{it}") for it in range(IT)]
    for it in range(IT):
        for jt in range(JT):
            nc.tensor.matmul(
                po1f[it][:], lhsT=adjT[jt][:, it * 128:(it + 1) * 128],
                rhs=arh65[jt][:], start=(jt == 0), stop=(jt == JT - 1),
            )
    p1sb = []
    for it in range(IT):
        t = big.tile([128, HEADS * C65], dt.float32, tag=f"p1sb_{it}")
        nc.scalar.copy(t[:], po1f[it][:])
        p1sb.append(t)

    osb2 = [tr.tile([C65, NB], dt.float32, name=f"osb2_{h}", tag=f"osb2_{h}")
            for h in range(HEADS)]
    for hpass in range(2):
        heads = (2 * hpass, 2 * hpass + 1)
        po2 = {h: pacc.tile([C65, NB], dt.float32, name=f"po2_{h}", tag=f"po2_{h % 2}")
               for h in heads}

        def emit_d(jt, h):
            dp = ps.tile([128, NB], dt.float32, tag="scr")
            nc.tensor.matmul(dp[:], lhsT=arbr[h][:, jt * 128:(jt + 1) * 128],
                             rhs=d_rhs[h][:], start=True, stop=True)
            return dp

        steps = [(jt, h) for jt in range(JT) for h in heads]
        dq = [emit_d(*steps[0])]
        for idx, (jt, h) in enumerate(steps):
            dp = dq.pop(0)
            if idx + 1 < len(steps):
                dq.append(emit_d(*steps[idx + 1]))
            r = tr.tile([128, NB], dt.bfloat16, tag="r")
            nc.vector.scalar_tensor_tensor(
                out=r[:], in0=dp[:], scalar=0.0, in1=adjT[jt][:],
                op0=Alu.max, op1=Alu.mult,
            )
            nc.tensor.matmul(
                po2[h][:], lhsT=h65[jt][:, h * C65:(h + 1) * C65], rhs=r[:],
                start=(jt == 0), stop=(jt == JT - 1),
            )
        for h in heads:
            nc.any.tensor_copy(osb2[h][:], po2[h][:])

    # ---- epilogue: transpose the residual, combine with flipped P1 ----------
    for it in range(IT):
        ot = tr.tile([128, HC], dt.float32, tag="ot")
        for h in range(HEADS):
            pt = ps.tile([128, C65], dt.float32, tag="scr")
            nc.tensor.transpose(
                pt[:], osb2[h][:, it * 128:(it + 1) * 128], idf[0:C65, 0:C65]
            )
            alc = al_cols[it][:, h:h + 1]
            num = tr.tile([128, OUT_C], dt.float32, tag="num")
            nc.vector.scalar_tensor_tensor(
                out=num[:], in0=p1sb[it][:, h * C65:h * C65 + OUT_C],
                scalar=alc, in1=pt[:, 0:OUT_C], op0=Alu.mult, op1=Alu.add,
            )
            dd = tr.tile([128, 1], dt.float32, tag="dd")
            nc.vector.scalar_tensor_tensor(
                out=dd[:], in0=p1sb[it][:, h * C65 + OUT_C:h * C65 + C65],
                scalar=alc, in1=pt[:, OUT_C:C65], op0=Alu.mult, op1=Alu.add,
            )
            rec = tr.tile([128, 1], dt.float32, tag="rec")
            nc.vector.reciprocal(rec[:], dd[:])
            nc.vector.scalar_tensor_tensor(
                out=ot[:, h * OUT_C:(h + 1) * OUT_C], in0=num[:],
                scalar=rec[:], in1=bias_b[:, h * OUT_C:(h + 1) * OUT_C],
                op0=Alu.mult, op1=Alu.add,
            )
        nc.sync.dma_start(out=out[it * 128:(it + 1) * 128, :], in_=ot[:])


def build():
    from contextlib import ExitStack
    import concourse.bacc as bacc
    import concourse.tile as tile
    from concourse import mybir

    dt = mybir.dt
    nc = bacc.Bacc("TRN2", target_bir_lowering=False, debug=False,
                   num_devices=NCORES)
    io = {
        "xT": nc.dram_tensor("xT", [IN_C, N], dt.float32r, kind="ExternalInput").ap(),
        "xoT": nc.dram_tensor("xoT", [IN_C, NB], dt.float32, kind="ExternalInput").ap(),
        "adjbT": nc.dram_tensor("adjbT", [N, NB], dt.int32, kind="ExternalInput").ap(),
        "Waug": nc.dram_tensor("Waug", [IN_C, HC + HEADS], dt.float32r, kind="ExternalInput").ap(),
        "Wal": nc.dram_tensor("Wal", [IN_C, HEADS], dt.float32, kind="ExternalInput").ap(),
        "bias": nc.dram_tensor("bias", [HC], dt.float32, kind="ExternalInput").ap(),
        "out": nc.dram_tensor("out", [NB, HC], dt.float32, kind="ExternalOutput").ap(),
    }
    with tile.TileContext(nc) as tc:
        with ExitStack() as ctx:
            _emit(ctx, tc, nc, io)
    nc.compile()
    return nc


def make_in_maps(x, adj, W, att_l, att_r, bias):
    x = np.asarray(x, np.float32)
    adj = np.ascontiguousarray(np.asarray(adj, np.int32))
    W = np.asarray(W, np.float32)
    att_l = np.asarray(att_l, np.float32)
    att_r = np.asarray(att_r, np.float32)
    bias = np.asarray(bias, np.float32)
    xT = np.ascontiguousarray(x.T)
    Wr = W.reshape(IN_C, HEADS, OUT_C)
    Wal = np.ascontiguousarray(np.einsum("khc,hc->kh", Wr, att_l))
    War = np.einsum("khc,hc->kh", Wr, att_r)
    Waug = np.ascontiguousarray(np.concatenate([W, War], axis=1))
    in_maps = []
    for m in range(NCORES):
        sl = slice(m * NB, (m + 1) * NB)
        in_maps.append({
            "xT": xT,
            "xoT": np.ascontiguousarray(x[sl].T),
            "adjbT": np.ascontiguousarray(adj[sl].T),
            "Waug": Waug,
            "Wal": Wal,
            "bias": bias,
        })
    return in_maps


def _install_ntff_shim():
    # this container image lacks antenv.axon_hooks; recreate it from the boot
    # helper so run_bass_kernel_spmd's trace path can find the profile hook
    import sys, types
    if "antenv.axon_hooks" in sys.modules:
        return
    from trn_agent_boot.trn_boot import _ntff_profile_via_ctypes
    hook = _ntff_profile_via_ctypes("/opt/axon/libaxon_pjrt.so")
    mod = types.ModuleType("antenv.axon_hooks")
    mod.get_axon_ntff_profile_hook = lambda: hook
    mod.set_axon_ntff_profile_hook = lambda h: None
    sys.modules["antenv.axon_hooks"] = mod


def kernel(x, adj, W, att_l, att_r, bias):
    from concourse.bass_utils import run_bass_kernel_spmd

    if "nc" not in _compiled:
        _compiled["nc"] = build()
    nc = _compiled["nc"]
    in_maps = make_in_maps(x, adj, W, att_l, att_r, bias)
    kwargs = {}
    if TRACE:
        _install_ntff_shim()
        kwargs["trace"] = True
    res = run_bass_kernel_spmd(nc, in_maps, core_ids=list(range(NCORES)), **kwargs)
    LAST_RESULTS["exec_time_ns"] = res.exec_time_ns
    LAST_RESULTS["mean_exec_time_ns"] = res.mean_exec_time_ns
    LAST_RESULTS["res"] = res
    return np.concatenate([res.results[m]["out"] for m in range(NCORES)], axis=0)



# revision 10
# speedup vs baseline: 1.1121x; 1.1121x over previous
# DenseGATConv on 8 Trainium2 NeuronCores (Bass/Tile, SPMD over destination rows).
#
# Math: h = x@W ; el/er = head-wise <h, att> ; e_ij = leaky(el_i + er_j) ;
#       alpha = softmax_j(mask(e)) ; out_i = sum_j alpha_ij h_j + bias.
# Key identity: exp(leaky(s)) = max(exp(s), exp(0.2 s)) since exp is monotone
# and leaky(s) = max(s, 0.2 s).  With s_ij = el_i + er_j both branches are
# rank-1 outer products: exp(s) = exp(el_i) exp(er_j).  The masked unnormalized
# attention is  pm[j,i] = adj[i,j] * max(al_i*ar_j, bl_i*br_j)  which needs no
# transcendentals on the [N,N,H] tensor — just two fused DVE ops + a max.
# The denominator rides along as a ones-column in the aggregation matmul.
#
# Sharding: destination rows i split across 8 cores (512 rows each); every core
# computes the full h (it needs all source nodes j anyway); params replicated.
import numpy as np

N, IN_C, HEADS, OUT_C = 4096, 256, 4, 64
HC = HEADS * OUT_C          # 256
NCORES = 8
NB = N // NCORES            # 512 destination rows per core
JT = N // 128               # 32 source-node tiles
IT = NB // 128              # 4 row subtiles per core
C65 = OUT_C + 1             # head slice + ones column

TRACE = False               # test.py flips this to collect HW exec time
LAST_RESULTS = {}           # exec_time_ns etc. stashed here when TRACE

_compiled = {}


def _emit(ctx, tc, nc, io):
    import concourse.bass as bass
    import concourse.masks as masks
    from concourse import mybir

    dt = mybir.dt
    Alu = mybir.AluOpType
    Act = mybir.ActivationFunctionType

    xT, xoT, adjbT, Waug, Wal, bias, out = (
        io["xT"], io["xoT"], io["adjbT"], io["Waug"], io["Wal"],
        io["bias"], io["out"],
    )
    bf = dt.bfloat16

    big = ctx.enter_context(tc.tile_pool(name="big", bufs=1))
    tr = ctx.enter_context(tc.tile_pool(name="tr", bufs=3))
    ps = ctx.enter_context(tc.tile_pool(name="ps", bufs=2, space="PSUM"))
    pf = ctx.enter_context(tc.tile_pool(name="pf", bufs=1, space="PSUM"))
    pacc = ctx.enter_context(tc.tile_pool(name="pacc", bufs=1, space="PSUM"))

    # ---- constants / params -------------------------------------------------
    idf = big.tile([128, 128], dt.float32, tag="idf")
    masks.make_identity(nc, idf[:])
    idb = big.tile([128, 128], dt.bfloat16, tag="idb")
    masks.make_identity(nc, idb[:])
    bias_b = big.tile([128, HC], dt.float32, tag="bias_b")
    bias_bcast_ap = bass.AP(
        tensor=bias.tensor, offset=bias.offset, ap=[[0, 128]] + list(bias.ap)
    )
    nc.gpsimd.dma_start(out=bias_b[:], in_=bias_bcast_ap)

    waug = []
    wal = []
    for ct in range(2):
        wg = big.tile([128, HC + HEADS], bf, tag=f"waug{ct}")
        nc.sync.dma_start(out=wg[:], in_=Waug[ct * 128:(ct + 1) * 128, :])
        waug.append(wg)
        wl = big.tile([128, HEADS], bf, tag=f"wal{ct}")
        nc.sync.dma_start(out=wl[:], in_=Wal[ct * 128:(ct + 1) * 128, :])
        wal.append(wl)

    xTr = []
    for ct in range(2):
        xf = big.tile([128, N], bf, tag=f"xTr{ct}")
        nc.sync.dma_start(out=xf[:], in_=xT[ct * 128:(ct + 1) * 128, :])
        xTr.append(xf)
    xo = []
    for ct in range(2):
        t = big.tile([128, NB], bf, tag=f"xoT{ct}")
        nc.sync.dma_start(out=t[:], in_=xoT[ct * 128:(ct + 1) * 128, :])
        xo.append(t)

    # ---- h65 (bf16 h + ones col) and er via one augmented matmul ------------
    # er_pack laid out h-major (col = h*32 + nt) so a PE transpose yields each
    # head's exp(er) as a 32-aligned partition block.
    h65 = []
    arh65 = []
    er_pack = big.tile([128, JT * HEADS], dt.float32, tag="er_pack")
    ar_pack = big.tile([128, JT * HEADS], dt.float32, tag="ar_pack")
    br_pack = big.tile([128, JT * HEADS], dt.float32, tag="br_pack")
    erp = er_pack[:].rearrange("p (h j) -> p h j", h=HEADS)
    for nt in range(JT):
        hps = ps.tile([128, HC + HEADS], dt.float32, tag="scr")
        for ct in range(2):
            nc.tensor.matmul(
                hps[:], lhsT=xTr[ct][:, nt * 128:(nt + 1) * 128], rhs=waug[ct][:],
                start=(ct == 0), stop=(ct == 1),
            )
        ht = big.tile([128, HEADS * C65], dt.bfloat16, tag=f"h65_{nt}")
        hr = ht[:].rearrange("p (h c) -> p h c", c=C65)
        hpr = hps[:, 0:HC].rearrange("p (h c) -> p h c", c=OUT_C)
        if nt % 2 == 0:
            nc.scalar.copy(hr[:, :, 0:OUT_C], hpr[:, :, :])
        else:
            nc.vector.tensor_copy(hr[:, :, 0:OUT_C], hpr[:, :, :])
        nc.vector.memset(hr[:, :, OUT_C], 1.0)
        h65.append(ht)
        nc.any.tensor_copy(erp[:, :, nt], hps[:, HC:HC + HEADS])
        if nt % 8 == 7:
            # exp the finished chunk: cols h*32+nt for nt in chunk, all h
            for h in range(HEADS):
                c0, c1 = h * JT + nt - 7, h * JT + nt + 1
                nc.scalar.activation(ar_pack[:, c0:c1], er_pack[:, c0:c1], Act.Exp)
                nc.scalar.activation(br_pack[:, c0:c1], er_pack[:, c0:c1],
                                     Act.Exp, scale=0.2)
            # ar-scaled copies of h65 (ar in the ones column -> denominator)
            for nt2 in range(nt - 7, nt + 1):
                at = big.tile([128, HEADS * C65], dt.bfloat16, tag=f"arh65_{nt2}")
                for h in range(HEADS):
                    sc = ar_pack[:, h * JT + nt2:h * JT + nt2 + 1]
                    nc.vector.tensor_scalar_mul(
                        at[:, h * C65:(h + 1) * C65],
                        h65[nt2][:, h * C65:(h + 1) * C65], sc,
                    )
                arh65.append(at)

    # transposed exp(er) rows per head: [2, N] bf16 (row0=br, row1=ar)
    arb16 = big.tile([128, JT * HEADS], dt.bfloat16, tag="arb16")
    brb16 = big.tile([128, JT * HEADS], dt.bfloat16, tag="brb16")
    nc.vector.tensor_copy(arb16[:], ar_pack[:])
    nc.vector.tensor_copy(brb16[:], br_pack[:])
    arT_ps = ps.tile([128, 128], dt.bfloat16, tag="scr")
    brT_ps = ps.tile([128, 128], dt.bfloat16, tag="scr")
    nc.tensor.transpose(arT_ps[:], arb16[:], idb[:])
    nc.tensor.transpose(brT_ps[:], brb16[:], idb[:])
    arT_sb = big.tile([128, 128], dt.bfloat16, tag="arT_sb")
    brT_sb = big.tile([128, 128], dt.bfloat16, tag="brT_sb")
    nc.vector.tensor_copy(arT_sb[:], arT_ps[:])
    nc.vector.tensor_copy(brT_sb[:], brT_ps[:])
    arbr = []
    for h in range(HEADS):
        t = big.tile([2, N], dt.bfloat16, tag=f"arbr_{h}", name=f"arbr_{h}")
        nc.sync.dma_start(out=t[0:1, :], in_=brT_sb[h * JT:(h + 1) * JT, :])
        nc.sync.dma_start(out=t[1:2, :], in_=arT_sb[h * JT:(h + 1) * JT, :])
        arbr.append(t)

    # ---- el side: exp rows + d-matmul rhs -----------------------------------
    d_rhs = []
    al_rows = []
    for h in range(HEADS):
        elp = ps.tile([1, NB], dt.float32, tag="scr")
        for ct in range(2):
            nc.tensor.matmul(
                elp[:], lhsT=wal[ct][:, h:h + 1], rhs=xo[ct][:],
                start=(ct == 0), stop=(ct == 1),
            )
        dr = big.tile([2, NB], dt.bfloat16, tag=f"d_rhs_{h}", name=f"d_rhs_{h}")
        # row0 = bl = exp(0.2 el) directly from ACT (partition 0 ok)
        nc.scalar.activation(dr[0:1, :], elp[:], Act.Exp, scale=0.2)
        al_row = big.tile([1, NB], dt.float32, tag=f"al_row_{h}")
        nc.scalar.activation(al_row[:], elp[:], Act.Exp)
        al_rows.append(al_row)
        nal = big.tile([1, NB], dt.bfloat16, tag=f"nal_{h}")
        nc.vector.tensor_scalar_mul(nal[:], al_row[:], -1.0)
        nc.sync.dma_start(out=dr[1:2, :], in_=nal[:])
        d_rhs.append(dr)

    al_cols = []
    for it in range(IT):
        t = big.tile([128, HEADS], dt.float32, tag=f"al_cols_{it}")
        for h in range(HEADS):
            nc.sync.dma_start(
                out=t[:, h:h + 1],
                in_=al_rows[h][0:1,
                    it * 128:(it + 1) * 128],
            )
        al_cols.append(t)

    # ---- adjacency: host-pretransposed+cast [N, NB] bf16; straight DMA ------
    adjT = []
    for jt in range(JT):
        ab = big.tile([128, NB], dt.bfloat16, tag=f"adjT{jt}", name=f"adjT{jt}")
        nc.sync.dma_start(out=ab[:], in_=adjbT[jt * 128:(jt + 1) * 128, :])
        adjT.append(ab)

    # ---- main loops: two head-passes (PSUM budget), mask rides the matmuls --
    # pm = m*A + m*relu(B-A):  the m*A term is a pure matmul (lhsT = ar-scaled
    # h65, rhs = adjacency); d = B-A comes from a K=2 rank-2 matmul; the only
    # per-element vector op is r = relu(d)*m (fused scalar_tensor_tensor).
    # flipped agg1: out[i, (h,c)] accumulators, one wide matmul per (jt, it);
    # lhsT = adjacency tile (i-slice), rhs = ar-scaled h65 for all heads.
    po1f = [pf.tile([128, HEADS * C65], dt.float32, name=f"po1f_{it}",
                    tag=f"po1f_{it}") for it in range(IT)]
    for it in range(IT):
        for jt in range(JT):
            nc.tensor.matmul(
                po1f[it][:], lhsT=adjT[jt][:, it * 128:(it + 1) * 128],
                rhs=arh65[jt][:], start=(jt == 0), stop=(jt == JT - 1),
            )
    p1sb = []
    for it in range(IT):
        t = big.tile([128, HEADS * C65], dt.float32, tag=f"p1sb_{it}")
        nc.scalar.copy(t[:], po1f[it][:])
        p1sb.append(t)

    osb2 = [tr.tile([C65, NB], dt.float32, name=f"osb2_{h}", tag=f"osb2_{h}")
            for h in range(HEADS)]
    for hpass in range(2):
        heads = (2 * hpass, 2 * hpass + 1)
        po2 = {h: pacc.tile([C65, NB], dt.float32, name=f"po2_{h}", tag=f"po2_{h % 2}")
               for h in heads}

        def emit_d(jt, h):
            dp = ps.tile([128, NB], dt.float32, tag="scr")
            nc.tensor.matmul(dp[:], lhsT=arbr[h][:, jt * 128:(jt + 1) * 128],
                             rhs=d_rhs[h][:], start=True, stop=True)
            return dp

        steps = [(jt, h) for jt in range(JT) for h in heads]
        dq = [emit_d(*steps[0])]
        for idx, (jt, h) in enumerate(steps):
            dp = dq.pop(0)
            if idx + 1 < len(steps):
                dq.append(emit_d(*steps[idx + 1]))
            r = tr.tile([128, NB], dt.bfloat16, tag="r")
            nc.vector.scalar_tensor_tensor(
                out=r[:], in0=dp[:], scalar=0.0, in1=adjT[jt][:],
                op0=Alu.max, op1=Alu.mult,
            )
            nc.tensor.matmul(
                po2[h][:], lhsT=h65[jt][:, h * C65:(h + 1) * C65], rhs=r[:],
                start=(jt == 0), stop=(jt == JT - 1),
            )
        for h in heads:
            nc.any.tensor_copy(osb2[h][:], po2[h][:])

    # ---- epilogue: transpose the residual, combine with flipped P1 ----------
    for it in range(IT):
        ot = tr.tile([128, HC], dt.float32, tag="ot")
        for h in range(HEADS):
            pt = ps.tile([128, C65], dt.float32, tag="scr")
            nc.tensor.transpose(
                pt[:], osb2[h][:, it * 128:(it + 1) * 128], idf[0:C65, 0:C65]
            )
            alc = al_cols[it][:, h:h + 1]
            num = tr.tile([128, OUT_C], dt.float32, tag="num")
            nc.vector.scalar_tensor_tensor(
                out=num[:], in0=p1sb[it][:, h * C65:h * C65 + OUT_C],
                scalar=alc, in1=pt[:, 0:OUT_C], op0=Alu.mult, op1=Alu.add,
            )
            dd = tr.tile([128, 1], dt.float32, tag="dd")
            nc.vector.scalar_tensor_tensor(
                out=dd[:], in0=p1sb[it][:, h * C65 + OUT_C:h * C65 + C65],
                scalar=alc, in1=pt[:, OUT_C:C65], op0=Alu.mult, op1=Alu.add,
            )
            rec = tr.tile([128, 1], dt.float32, tag="rec")
            nc.vector.reciprocal(rec[:], dd[:])
            nc.vector.scalar_tensor_tensor(
                out=ot[:, h * OUT_C:(h + 1) * OUT_C], in0=num[:],
                scalar=rec[:], in1=bias_b[:, h * OUT_C:(h + 1) * OUT_C],
                op0=Alu.mult, op1=Alu.add,
            )
        nc.sync.dma_start(out=out[it * 128:(it + 1) * 128, :], in_=ot[:])


def build():
    from contextlib import ExitStack
    import concourse.bacc as bacc
    import concourse.tile as tile
    from concourse import mybir

    dt = mybir.dt
    nc = bacc.Bacc("TRN2", target_bir_lowering=False, debug=False,
                   num_devices=NCORES)
    bf = dt.bfloat16
    io = {
        "xT": nc.dram_tensor("xT", [IN_C, N], bf, kind="ExternalInput").ap(),
        "xoT": nc.dram_tensor("xoT", [IN_C, NB], bf, kind="ExternalInput").ap(),
        "adjbT": nc.dram_tensor("adjbT", [N, NB], bf, kind="ExternalInput").ap(),
        "Waug": nc.dram_tensor("Waug", [IN_C, HC + HEADS], bf, kind="ExternalInput").ap(),
        "Wal": nc.dram_tensor("Wal", [IN_C, HEADS], bf, kind="ExternalInput").ap(),
        "bias": nc.dram_tensor("bias", [HC], dt.float32, kind="ExternalInput").ap(),
        "out": nc.dram_tensor("out", [NB, HC], dt.float32, kind="ExternalOutput").ap(),
    }
    with tile.TileContext(nc) as tc:
        with ExitStack() as ctx:
            _emit(ctx, tc, nc, io)
    nc.compile()
    return nc


def make_in_maps(x, adj, W, att_l, att_r, bias):
    import ml_dtypes
    bf16 = ml_dtypes.bfloat16
    x = np.asarray(x, np.float32)
    adj = np.asarray(adj, np.int32)
    W = np.asarray(W, np.float32)
    att_l = np.asarray(att_l, np.float32)
    att_r = np.asarray(att_r, np.float32)
    bias = np.asarray(bias, np.float32)
    xT = np.ascontiguousarray(x.T.astype(bf16))
    Wr = W.reshape(IN_C, HEADS, OUT_C)
    Wal = np.ascontiguousarray(
        np.einsum("khc,hc->kh", Wr, att_l).astype(bf16))
    War = np.einsum("khc,hc->kh", Wr, att_r)
    Waug = np.ascontiguousarray(
        np.concatenate([W, War], axis=1).astype(bf16))
    adjb = adj.astype(bf16)
    in_maps = []
    for m in range(NCORES):
        sl = slice(m * NB, (m + 1) * NB)
        in_maps.append({
            "xT": xT,
            "xoT": np.ascontiguousarray(x[sl].T.astype(bf16)),
            "adjbT": np.ascontiguousarray(adjb[sl].T),
            "Waug": Waug,
            "Wal": Wal,
            "bias": bias,
        })
    return in_maps


def _install_ntff_shim():
    # this container image lacks antenv.axon_hooks; recreate it from the boot
    # helper so run_bass_kernel_spmd's trace path can find the profile hook
    import sys, types
    if "antenv.axon_hooks" in sys.modules:
        return
    from trn_agent_boot.trn_boot import _ntff_profile_via_ctypes
    hook = _ntff_profile_via_ctypes("/opt/axon/libaxon_pjrt.so")
    mod = types.ModuleType("antenv.axon_hooks")
    mod.get_axon_ntff_profile_hook = lambda: hook
    mod.set_axon_ntff_profile_hook = lambda h: None
    sys.modules["antenv.axon_hooks"] = mod


def kernel(x, adj, W, att_l, att_r, bias):
    from concourse.bass_utils import run_bass_kernel_spmd

    if "nc" not in _compiled:
        _compiled["nc"] = build()
    nc = _compiled["nc"]
    in_maps = make_in_maps(x, adj, W, att_l, att_r, bias)
    kwargs = {}
    if TRACE:
        _install_ntff_shim()
        kwargs["trace"] = True
    res = run_bass_kernel_spmd(nc, in_maps, core_ids=list(range(NCORES)), **kwargs)
    LAST_RESULTS["exec_time_ns"] = res.exec_time_ns
    LAST_RESULTS["mean_exec_time_ns"] = res.mean_exec_time_ns
    LAST_RESULTS["res"] = res
    return np.concatenate([res.results[m]["out"] for m in range(NCORES)], axis=0)



# revision 12
# speedup vs baseline: 1.1215x; 1.0084x over previous
# DenseGATConv on 8 Trainium2 NeuronCores (Bass/Tile, SPMD over destination rows).
#
# Math: h = x@W ; el/er = head-wise <h, att> ; e_ij = leaky(el_i + er_j) ;
#       alpha = softmax_j(mask(e)) ; out_i = sum_j alpha_ij h_j + bias.
# Key identity: exp(leaky(s)) = max(exp(s), exp(0.2 s)), and with
# s_ij = el_i + er_j both branches are rank-1:  pm = adj * max(al_i ar_j,
# bl_i br_j) = adj*al_i*ar_j + m2*(bl_i br_j - al_i ar_j)  where
# m2 = adj * [s < 0] is a BINARY mask.  So the whole [N,N,H] nonlinearity
# reduces to producing m2 with two cheap SBUF vector ops (compare + mult
# against a broadcast row of -el), and the output is three adjacency-style
# aggregations:  S0 = adj @ (ar h | ar),  S1 = m2 @ (ar h | ar),
# S2 = m2 @ (br h | br);  out = (al(S0-S1) + bl S2)[:, :64] / denom + bias.
# S1 and S2 share the mask as stationary weights: one 130-col output-
# stationary matmul per (it, jt, head) — no [N,N,H] tensor ever hits PE
# twice, and the epilogue needs no transposes (output is i-partitioned).
#
# Sharding: destination rows i split across 8 cores (512 rows each); every core
# computes the full h (it needs all source nodes j anyway); params replicated.
import numpy as np

N, IN_C, HEADS, OUT_C = 4096, 256, 4, 64
HC = HEADS * OUT_C          # 256
NCORES = 8
NB = N // NCORES            # 512 destination rows per core
JT = N // 128               # 32 source-node tiles
IT = NB // 128              # 4 row subtiles per core
C65 = OUT_C + 1             # head slice + ones column

TRACE = False               # test.py flips this to collect HW exec time
LAST_RESULTS = {}           # exec_time_ns etc. stashed here when TRACE

_compiled = {}


def _emit(ctx, tc, nc, io):
    import concourse.bass as bass
    import concourse.masks as masks
    from concourse import mybir

    dt = mybir.dt
    Alu = mybir.AluOpType
    Act = mybir.ActivationFunctionType

    xT, xoT, adjbT, Waug, Wal, bias, out = (
        io["xT"], io["xoT"], io["adjbT"], io["Waug"], io["Wal"],
        io["bias"], io["out"],
    )
    bf = dt.bfloat16

    big = ctx.enter_context(tc.tile_pool(name="big", bufs=1))
    tr = ctx.enter_context(tc.tile_pool(name="tr", bufs=3))
    ps = ctx.enter_context(tc.tile_pool(name="ps", bufs=2, space="PSUM"))
    pf = ctx.enter_context(tc.tile_pool(name="pf", bufs=1, space="PSUM"))

    # ---- constants / params -------------------------------------------------
    bias_b = big.tile([128, HC], dt.float32, tag="bias_b")
    bias_bcast_ap = bass.AP(
        tensor=bias.tensor, offset=bias.offset, ap=[[0, 128]] + list(bias.ap)
    )
    nc.gpsimd.dma_start(out=bias_b[:], in_=bias_bcast_ap)

    waug = []
    wal = []
    for ct in range(2):
        wg = big.tile([128, HC + HEADS], bf, tag=f"waug{ct}")
        nc.sync.dma_start(out=wg[:], in_=Waug[ct * 128:(ct + 1) * 128, :])
        waug.append(wg)
        wl = big.tile([128, HEADS], bf, tag=f"wal{ct}")
        nc.sync.dma_start(out=wl[:], in_=Wal[ct * 128:(ct + 1) * 128, :])
        wal.append(wl)

    xTr = []
    for ct in range(2):
        xf = big.tile([128, N], bf, tag=f"xTr{ct}")
        nc.sync.dma_start(out=xf[:], in_=xT[ct * 128:(ct + 1) * 128, :])
        xTr.append(xf)
    xo = []
    for ct in range(2):
        t = big.tile([128, NB], bf, tag=f"xoT{ct}")
        nc.sync.dma_start(out=t[:], in_=xoT[ct * 128:(ct + 1) * 128, :])
        xo.append(t)

    # ---- h65 (bf16 h + ones col) and er via one augmented matmul ------------
    # er_pack laid out h-major (col = h*32 + nt) so a PE transpose yields each
    # head's exp(er) as a 32-aligned partition block.
    h65 = []
    arh65 = []
    brh65 = []
    er_pack = big.tile([128, JT * HEADS], dt.float32, tag="er_pack")
    ar_pack = big.tile([128, JT * HEADS], dt.float32, tag="ar_pack")
    br_pack = big.tile([128, JT * HEADS], dt.float32, tag="br_pack")
    erp = er_pack[:].rearrange("p (h j) -> p h j", h=HEADS)
    for nt in range(JT):
        hps = ps.tile([128, HC + HEADS], dt.float32, tag="scr")
        for ct in range(2):
            nc.tensor.matmul(
                hps[:], lhsT=xTr[ct][:, nt * 128:(nt + 1) * 128], rhs=waug[ct][:],
                start=(ct == 0), stop=(ct == 1),
            )
        ht = big.tile([128, HEADS * C65], dt.bfloat16, tag=f"h65_{nt}")
        hr = ht[:].rearrange("p (h c) -> p h c", c=C65)
        hpr = hps[:, 0:HC].rearrange("p (h c) -> p h c", c=OUT_C)
        if nt % 2 == 0:
            nc.scalar.copy(hr[:, :, 0:OUT_C], hpr[:, :, :])
        else:
            nc.vector.tensor_copy(hr[:, :, 0:OUT_C], hpr[:, :, :])
        nc.vector.memset(hr[:, :, OUT_C], 1.0)
        h65.append(ht)
        nc.any.tensor_copy(erp[:, :, nt], hps[:, HC:HC + HEADS])
        if nt % 8 == 7:
            # exp the finished chunk: cols h*32+nt for nt in chunk, all h
            for h in range(HEADS):
                c0, c1 = h * JT + nt - 7, h * JT + nt + 1
                nc.scalar.activation(ar_pack[:, c0:c1], er_pack[:, c0:c1], Act.Exp)
                nc.scalar.activation(br_pack[:, c0:c1], er_pack[:, c0:c1],
                                     Act.Exp, scale=0.2)
            # ar-scaled copies of h65 (agg1 rhs, 260-contiguous) and the
            # per-head [ar h|ar | br h|br] 130-col blocks (masked-agg rhs)
            for nt2 in range(nt - 7, nt + 1):
                at = big.tile([128, HEADS * C65], dt.bfloat16, tag=f"arh65_{nt2}")
                vt = big.tile([128, HEADS * 2 * C65], dt.bfloat16,
                              tag=f"vh130_{nt2}")
                for h in range(HEADS):
                    sc = ar_pack[:, h * JT + nt2:h * JT + nt2 + 1]
                    hb = h65[nt2][:, h * C65:(h + 1) * C65]
                    nc.vector.tensor_scalar_mul(
                        at[:, h * C65:(h + 1) * C65], hb, sc)
                    nc.vector.tensor_scalar_mul(
                        vt[:, h * 2 * C65:h * 2 * C65 + C65], hb, sc)
                    sb = br_pack[:, h * JT + nt2:h * JT + nt2 + 1]
                    nc.vector.tensor_scalar_mul(
                        vt[:, h * 2 * C65 + C65:(h + 1) * 2 * C65], hb, sb)
                arh65.append(at)
                brh65.append(vt)

    # ---- el side: -el broadcast tiles (mask compares) + al/bl columns -------
    nel_b = []
    al_rows = []
    bl_rows = []
    for h in range(HEADS):
        elp = ps.tile([1, NB], dt.float32, tag="scr")
        for ct in range(2):
            nc.tensor.matmul(
                elp[:], lhsT=wal[ct][:, h:h + 1], rhs=xo[ct][:],
                start=(ct == 0), stop=(ct == 1),
            )
        nel_row = big.tile([1, NB], dt.bfloat16, tag=f"nel_row_{h}")
        nc.scalar.activation(nel_row[:], elp[:], Act.Copy, scale=-1.0)
        al_row = big.tile([1, NB], dt.float32, tag=f"al_row_{h}")
        nc.scalar.activation(al_row[:], elp[:], Act.Exp)
        al_rows.append(al_row)
        bl_row = big.tile([1, NB], dt.float32, tag=f"bl_row_{h}")
        nc.scalar.activation(bl_row[:], elp[:], Act.Exp, scale=0.2)
        bl_rows.append(bl_row)
        nb_t = big.tile([128, NB], dt.bfloat16, tag=f"nel_b_{h}")
        nc.gpsimd.partition_broadcast(nb_t[:], nel_row[:])
        nel_b.append(nb_t)

    al_cols = []
    bl_cols = []
    for it in range(IT):
        t = big.tile([128, HEADS], dt.float32, tag=f"al_cols_{it}")
        t2 = big.tile([128, HEADS], dt.float32, tag=f"bl_cols_{it}")
        for h in range(HEADS):
            nc.sync.dma_start(
                out=t[:, h:h + 1],
                in_=al_rows[h][0:1, it * 128:(it + 1) * 128],
            )
            nc.sync.dma_start(
                out=t2[:, h:h + 1],
                in_=bl_rows[h][0:1, it * 128:(it + 1) * 128],
            )
        al_cols.append(t)
        bl_cols.append(t2)

    # ---- adjacency: host-pretransposed+cast [N, NB] bf16; straight DMA ------
    adjT = []
    for jt in range(JT):
        ab = big.tile([128, NB], dt.bfloat16, tag=f"adjT{jt}", name=f"adjT{jt}")
        nc.sync.dma_start(out=ab[:], in_=adjbT[jt * 128:(jt + 1) * 128, :])
        adjT.append(ab)

    # ---- main loops: two head-passes (PSUM budget), mask rides the matmuls --
    # pm = m*A + m*relu(B-A):  the m*A term is a pure matmul (lhsT = ar-scaled
    # h65, rhs = adjacency); d = B-A comes from a K=2 rank-2 matmul; the only
    # per-element vector op is r = relu(d)*m (fused scalar_tensor_tensor).
    # flipped agg1: out[i, (h,c)] accumulators, one wide matmul per (jt, it);
    # lhsT = adjacency tile (i-slice), rhs = ar-scaled h65 for all heads.
    po1f = [pf.tile([128, HEADS * C65], dt.float32, name=f"po1f_{it}",
                    tag=f"po1f_{it}") for it in range(IT)]
    for it in range(IT):
        for jt in range(JT):
            nc.tensor.matmul(
                po1f[it][:], lhsT=adjT[jt][:, it * 128:(it + 1) * 128],
                rhs=arh65[jt][:], start=(jt == 0), stop=(jt == JT - 1),
            )
    p1sb = []
    for it in range(IT):
        t = big.tile([128, HEADS * C65], dt.float32, tag=f"p1sb_{it}")
        nc.scalar.copy(t[:], po1f[it][:])
        p1sb.append(t)

    # ---- masked residual: S1/S2 = m2 @ [ar h|ar | br h|br], m2 stationary ---
    # pm2 pair tiles reuse the po1f PSUM banks (same tag/shape); pair p holds
    # it = 2p, 2p+1 as two 130-col accumulation groups in one bank (only the
    # bank's first matmul at jt=0 carries start=True).
    ot = [big.tile([128, HC], dt.float32, tag=f"ot_{it}", name=f"ot_{it}")
          for it in range(IT)]
    W2 = 2 * C65
    for h in range(HEADS):
        pm2 = [pf.tile([128, HEADS * C65], dt.float32,
                       tag=f"po1f_{2 * (h % 2) + p}", name=f"pm2_{h}_{p}")
               for p in range(2)]
        for jt in range(JT):
            ecol = er_pack[:, h * JT + jt:h * JT + jt + 1]
            m2 = tr.tile([128, NB], dt.bfloat16, tag="m2")
            tmp = tr.tile([128, NB], dt.bfloat16, tag="m2t")
            nc.vector.tensor_scalar(tmp[:], nel_b[h][:], ecol, None,
                                    Alu.is_gt)
            if jt % 3 == 2:
                nc.gpsimd.tensor_mul(m2[:], tmp[:], adjT[jt][:])
            else:
                nc.vector.tensor_mul(m2[:], tmp[:], adjT[jt][:])
            for it in range(IT):
                pt = pm2[it // 2]
                c0 = (it % 2) * W2
                nc.tensor.matmul(
                    pt[:, c0:c0 + W2], lhsT=m2[:, it * 128:(it + 1) * 128],
                    rhs=brh65[jt][:, h * W2:(h + 1) * W2],
                    start=(jt == 0 and it % 2 == 0), stop=(jt == JT - 1),
                    skip_group_check=True,
                )
        # drain head h: out block = (al*(S0-S1) + bl*S2) / denom + bias
        for it in range(IT):
            pt = pm2[it // 2]
            c0 = (it % 2) * W2
            alc = al_cols[it][:, h:h + 1]
            blc = bl_cols[it][:, h:h + 1]
            e1 = tr.tile([128, C65], dt.bfloat16, tag="e1")
            nc.vector.tensor_sub(
                e1[:], p1sb[it][:, h * C65:(h + 1) * C65], pt[:, c0:c0 + C65])
            e2 = tr.tile([128, C65], dt.bfloat16, tag="e2")
            nc.vector.tensor_scalar_mul(e2[:], e1[:], alc)
            cmb = tr.tile([128, C65], dt.float32, tag="cmb")
            nc.vector.scalar_tensor_tensor(
                out=cmb[:], in0=pt[:, c0 + C65:c0 + W2], scalar=blc,
                in1=e2[:], op0=Alu.mult, op1=Alu.add,
            )
            rec = tr.tile([128, 1], dt.float32, tag="rec")
            nc.vector.reciprocal(rec[:], cmb[:, OUT_C:C65])
            nc.vector.scalar_tensor_tensor(
                out=ot[it][:, h * OUT_C:(h + 1) * OUT_C], in0=cmb[:, 0:OUT_C],
                scalar=rec[:], in1=bias_b[:, h * OUT_C:(h + 1) * OUT_C],
                op0=Alu.mult, op1=Alu.add,
            )
    for it in range(IT):
        nc.sync.dma_start(out=out[it * 128:(it + 1) * 128, :], in_=ot[it][:])


def build():
    from contextlib import ExitStack
    import concourse.bacc as bacc
    import concourse.tile as tile
    from concourse import mybir

    dt = mybir.dt
    nc = bacc.Bacc("TRN2", target_bir_lowering=False, debug=False,
                   num_devices=NCORES)
    bf = dt.bfloat16
    io = {
        "xT": nc.dram_tensor("xT", [IN_C, N], bf, kind="ExternalInput").ap(),
        "xoT": nc.dram_tensor("xoT", [IN_C, NB], bf, kind="ExternalInput").ap(),
        "adjbT": nc.dram_tensor("adjbT", [N, NB], bf, kind="ExternalInput").ap(),
        "Waug": nc.dram_tensor("Waug", [IN_C, HC + HEADS], bf, kind="ExternalInput").ap(),
        "Wal": nc.dram_tensor("Wal", [IN_C, HEADS], bf, kind="ExternalInput").ap(),
        "bias": nc.dram_tensor("bias", [HC], dt.float32, kind="ExternalInput").ap(),
        "out": nc.dram_tensor("out", [NB, HC], dt.float32, kind="ExternalOutput").ap(),
    }
    with tile.TileContext(nc) as tc:
        with ExitStack() as ctx:
            _emit(ctx, tc, nc, io)
    nc.compile()
    return nc


def make_in_maps(x, adj, W, att_l, att_r, bias):
    import ml_dtypes
    bf16 = ml_dtypes.bfloat16
    x = np.asarray(x, np.float32)
    adj = np.asarray(adj, np.int32)
    W = np.asarray(W, np.float32)
    att_l = np.asarray(att_l, np.float32)
    att_r = np.asarray(att_r, np.float32)
    bias = np.asarray(bias, np.float32)
    xT = np.ascontiguousarray(x.T.astype(bf16))
    Wr = W.reshape(IN_C, HEADS, OUT_C)
    Wal = np.ascontiguousarray(
        np.einsum("khc,hc->kh", Wr, att_l).astype(bf16))
    War = np.einsum("khc,hc->kh", Wr, att_r)
    Waug = np.ascontiguousarray(
        np.concatenate([W, War], axis=1).astype(bf16))
    adjb = adj.astype(bf16)
    in_maps = []
    for m in range(NCORES):
        sl = slice(m * NB, (m + 1) * NB)
        in_maps.append({
            "xT": xT,
            "xoT": np.ascontiguousarray(x[sl].T.astype(bf16)),
            "adjbT": np.ascontiguousarray(adjb[sl].T),
            "Waug": Waug,
            "Wal": Wal,
            "bias": bias,
        })
    return in_maps


def _install_ntff_shim():
    # this container image lacks antenv.axon_hooks; recreate it from the boot
    # helper so run_bass_kernel_spmd's trace path can find the profile hook
    import sys, types
    if "antenv.axon_hooks" in sys.modules:
        return
    from trn_agent_boot.trn_boot import _ntff_profile_via_ctypes
    hook = _ntff_profile_via_ctypes("/opt/axon/libaxon_pjrt.so")
    mod = types.ModuleType("antenv.axon_hooks")
    mod.get_axon_ntff_profile_hook = lambda: hook
    mod.set_axon_ntff_profile_hook = lambda h: None
    sys.modules["antenv.axon_hooks"] = mod


def kernel(x, adj, W, att_l, att_r, bias):
    from concourse.bass_utils import run_bass_kernel_spmd

    if "nc" not in _compiled:
        _compiled["nc"] = build()
    nc = _compiled["nc"]
    in_maps = make_in_maps(x, adj, W, att_l, att_r, bias)
    kwargs = {}
    if TRACE:
        _install_ntff_shim()
        kwargs["trace"] = True
    res = run_bass_kernel_spmd(nc, in_maps, core_ids=list(range(NCORES)), **kwargs)
    LAST_RESULTS["exec_time_ns"] = res.exec_time_ns
    LAST_RESULTS["mean_exec_time_ns"] = res.mean_exec_time_ns
    LAST_RESULTS["res"] = res
    return np.concatenate([res.results[m]["out"] for m in range(NCORES)], axis=0)

